# revision 1
# baseline (speedup 1.0000x reference)
"""CGCNN DOS predictor on 8 trn2 NeuronCores (Bass/Tile SPMD kernel).

Sharding: dst-partitioned edges; nodes permuted into 8 cores x 99 windows
x 128 slots with balanced per-window edge counts (<= 2048). h replicated
(bf16) on every core; per layer one AllGather moves raw aggregates + BN
stat partials, and every core redundantly applies BN + residual so h stays
consistent with no further communication.

msg = sigmoid(F)*softplus(S) runs entirely off the {Exp, Ln} ACT table by
storing [-F | S]: e1 = exp(.); l1 = ln(e1 + 1) = [sp(-F) | sp(S)];
sigmoid(F) = exp(-sp(-F)); msg = sigmoid * sp(S).
"""
import os
import sys

sys.path.insert(0, "/opt/trn_rl_repo")

import numpy as np
import ml_dtypes

import concourse.bass as bass
import concourse.bacc as bacc
import concourse.tile as tile
from concourse import mybir
from concourse.bass_utils import run_bass_kernel_spmd

N_NODES = 100000
NUM_GRAPHS = 128
ATOM_IN = 92
FEA = 64
EDGE_DIM = 41
N_CONV = 5
HID = 256
LATENT = 128
CH = 3
BN_EPS = 1e-5

P = 128
NCORES = 8
NW = 99                 # windows per core
TPW = 16                # edge tiles per window
VC = NW * P             # 12672 node slots per core
VTOT = NCORES * VC      # 101376
EC = NW * TPW * P       # 202752 edge slots per core
NT = NW * TPW           # 1584 tiles per core
NCH = VTOT // P         # 792 chunks in the replicated node table

f32 = mybir.dt.float32
bf16 = mybir.dt.bfloat16
i32 = mybir.dt.int32
AF = mybir.ActivationFunctionType
OP = mybir.AluOpType

_cache = {}


# ======================== device kernel ========================

def _build():
    nc = bacc.Bacc("TRN2", target_bir_lowering=False, debug=False,
                   num_devices=NCORES)

    ext_in = lambda n, s, d: nc.dram_tensor(n, s, d, kind="ExternalInput")
    xT_in = ext_in("xT_in", [ATOM_IN, VC], bf16)
    src_in = ext_in("src_in", [NW, P, TPW], i32)
    slot_in = ext_in("slot_in", [NW, P, TPW], bf16)
    dstloc_in = ext_in("dstloc_in", [NW, P, TPW], i32)
    eaT_in = ext_in("eaT_in", [EDGE_DIM, EC], bf16)
    ownrows_in = ext_in("ownrows_in", [NW, P], i32)
    maskown_in = ext_in("maskown_in", [P, NW], bf16)
    batch_in = ext_in("batch_in", [P, NCH], bf16)
    mask_in = ext_in("mask_in", [P, NCH], bf16)
    iota_in = ext_in("iota_in", [P, P], bf16)
    embW_in = ext_in("embW_in", [ATOM_IN, FEA], bf16)
    embB_in = ext_in("embB_in", [P, FEA], f32)
    Wi_in = ext_in("Wi_in", [64, N_CONV * 256], bf16)
    Wj_in = ext_in("Wj_in", [64, N_CONV * 256], bf16)
    Wea_in = ext_in("Wea_in", [EDGE_DIM, N_CONV * 256], bf16)
    bias_in = ext_in("bias_in", [P, N_CONV * 256], bf16)
    gam_in = ext_in("gam_in", [1, N_CONV * 256], f32)   # gamma at +0
    bet_in = ext_in("bet_in", [1, N_CONV * 256], f32)   # beta at +0
    W1_in = ext_in("W1_in", [FEA, HID], bf16)
    W2_in = ext_in("W2_in", [HID, HID], bf16)
    W3_in = ext_in("W3_in", [HID, LATENT * CH], bf16)
    b1_in = ext_in("b1_in", [P, HID], f32)
    b2_in = ext_in("b2_in", [P, HID], f32)
    b3_in = ext_in("b3_in", [P, LATENT * CH], f32)

    dos_out = nc.dram_tensor("dos_out", [P, LATENT * CH], f32,
                             kind="ExternalOutput")
    hdbg_out = nc.dram_tensor("hdbg_out", [P, FEA], f32, kind="ExternalOutput")

    bfs_all = nc.dram_tensor("bfs_all", [VTOT, P], bf16)
    afs_dram = nc.dram_tensor("afs_dram", [VC, P], bf16)
    h_dram = nc.dram_tensor("h_dram", [VTOT, FEA], bf16)
    h0_bounce = nc.dram_tensor("h0_bounce", [VC, FEA], bf16)
    h0_all = nc.dram_tensor("h0_all", [VTOT, FEA], bf16, addr_space="Shared")
    agg_bounce = nc.dram_tensor("agg_bounce", [VC, FEA], bf16)
    stats_bounce = nc.dram_tensor("stats_bounce", [1, P], f32)
    agg_all = nc.dram_tensor("agg_all", [VTOT, FEA], bf16, addr_space="Shared")
    stats_all = nc.dram_tensor("stats_all", [NCORES, P], f32,
                               addr_space="Shared")
    rg = [list(range(NCORES))]

    with tile.TileContext(nc) as tc:
        with (
            tc.tile_pool(name="const", bufs=1) as cp,
            tc.tile_pool(name="work", bufs=3) as wp,
            tc.tile_pool(name="gath", bufs=4) as gp,
            tc.tile_pool(name="widx", bufs=3) as wi_p,
            tc.tile_pool(name="fs", bufs=3, space="PSUM") as fsp,
            tc.tile_pool(name="agg", bufs=2, space="PSUM") as agp,
            tc.tile_pool(name="aux", bufs=2, space="PSUM") as axp,
            tc.tile_pool(name="acc1", bufs=1, space="PSUM") as a1p,
        ):
            # ---- persistent SBUF ----
            h_all = cp.tile([P, NCH * FEA], bf16, tag="h_all")
            iota = cp.tile([P, P], bf16, tag="iota")
            ident = cp.tile([P, P], bf16, tag="ident")
            identf = cp.tile([P, P], f32, tag="identf")
            batch_sb = cp.tile([P, NCH], bf16, tag="batch")
            mask_sb = cp.tile([P, NCH], bf16, tag="mask")
            maskown_sb = cp.tile([P, NW], bf16, tag="maskown")
            Wi_sb = cp.tile([64, N_CONV * 256], bf16, tag="Wi")
            Wj_sb = cp.tile([64, N_CONV * 256], bf16, tag="Wj")
            Wea_sb = cp.tile([EDGE_DIM, N_CONV * 256], bf16, tag="Wea")
            bias_sb = cp.tile([P, N_CONV * 256], bf16, tag="bias")
            gam_sb = cp.tile([1, N_CONV * 256], f32, tag="gam")
            bet_sb = cp.tile([1, N_CONV * 256], f32, tag="bet")
            embW_sb = cp.tile([ATOM_IN, FEA], bf16, tag="embW")
            embB_sb = cp.tile([P, FEA], f32, tag="embB")
            ones8 = cp.tile([NCORES, 1], f32, tag="ones8")
            W1_sb = cp.tile([FEA, HID], bf16, tag="W1")
            W2a_sb = cp.tile([P, HID], bf16, tag="W2a")
            W2b_sb = cp.tile([P, HID], bf16, tag="W2b")
            W3a_sb = cp.tile([P, LATENT * CH], bf16, tag="W3a")
            W3b_sb = cp.tile([P, LATENT * CH], bf16, tag="W3b")
            b1_sb = cp.tile([P, HID], f32, tag="b1")
            b2_sb = cp.tile([P, HID], f32, tag="b2")
            b3_sb = cp.tile([P, LATENT * CH], f32, tag="b3")

            for t_, s_ in [(iota, iota_in), (batch_sb, batch_in),
                           (mask_sb, mask_in), (maskown_sb, maskown_in),
                           (Wi_sb, Wi_in), (Wj_sb, Wj_in), (Wea_sb, Wea_in),
                           (bias_sb, bias_in), (gam_sb, gam_in),
                           (bet_sb, bet_in),
                           (embW_sb, embW_in), (embB_sb, embB_in),
                           (W1_sb, W1_in), (b1_sb, b1_in), (b2_sb, b2_in),
                           (b3_sb, b3_in)]:
                nc.sync.dma_start(out=t_[:], in_=s_[:])
            nc.sync.dma_start(out=W2a_sb[:], in_=W2_in[0:P, :])
            nc.sync.dma_start(out=W2b_sb[:], in_=W2_in[P:HID, :])
            nc.sync.dma_start(out=W3a_sb[:], in_=W3_in[0:P, :])
            nc.sync.dma_start(out=W3b_sb[:], in_=W3_in[P:HID, :])
            nc.gpsimd.memset(ones8[:], 1.0)
            eps_t = cp.tile([1, 1], f32, tag="eps")
            nc.gpsimd.memset(eps_t[:], BN_EPS)
            from concourse.masks import make_identity
            make_identity(nc, ident[:])
            make_identity(nc, identf[:])

            # ================= Phase 0: embedding =================
            for w in range(NW):
                xw = wp.tile([ATOM_IN, P], bf16, tag="xw")
                nc.sync.dma_start(out=xw[:], in_=xT_in[:, w * P:(w + 1) * P])
                pe = axp.tile([P, FEA], f32, space="PSUM", tag="tp")
                nc.tensor.matmul(out=pe[:], lhsT=xw[:],
                                 rhs=embW_sb[:], start=True, stop=True)
                ht = wp.tile([P, FEA], bf16, tag="h0")
                nc.vector.tensor_tensor(out=ht[:], in0=pe[:], in1=embB_sb[:],
                                        op=OP.add)
                nc.sync.dma_start(out=h0_bounce[w * P:(w + 1) * P, :],
                                  in_=ht[:])
            nc.gpsimd.collective_compute(
                "AllGather", OP.bypass, replica_groups=rg,
                ins=[h0_bounce[:]], outs=[h0_all[:]])
            for ch in range(NCH):
                nc.sync.dma_start(out=h_all[:, ch * FEA:(ch + 1) * FEA],
                                  in_=h0_all[ch * P:(ch + 1) * P, :])
            nc.sync.dma_start(out=h_dram[:], in_=h0_all[:])

            # ================= layers =================
            for lv in range(0, N_CONV * 256, 256):
                # ---- A1: bfs_all projections for every chunk ----
                for ch in range(NCH):
                    tp = axp.tile([FEA, P], bf16, space="PSUM", tag="tp")
                    nc.tensor.transpose(
                        out=tp[:], in_=h_all[:, ch * FEA:(ch + 1) * FEA],
                        identity=ident[:])
                    hT = wp.tile([FEA, P], bf16, tag="hT")
                    nc.vector.tensor_copy(out=hT[:], in_=tp[:])
                    pb = axp.tile([P, P], f32, space="PSUM", tag="tp")
                    nc.tensor.matmul(out=pb[:], lhsT=hT[:],
                                     rhs=Wj_sb[:, bass.ds(lv, 128)],
                                     start=True, stop=True)
                    bt = wp.tile([P, P], bf16, tag="bfs")
                    nc.vector.tensor_copy(out=bt[:], in_=pb[:])
                    nc.sync.dma_start(out=bfs_all[ch * P:(ch + 1) * P, :],
                                      in_=bt[:])

                # ---- A2: afs for own windows ----
                for w in range(NW):
                    orow = wi_p.tile([P, 1], i32, tag="orow")
                    nc.sync.dma_start(out=orow[:], in_=ownrows_in[w, :, None])
                    hw = gp.tile([P, FEA], bf16, tag="hw")
                    nc.gpsimd.indirect_dma_start(
                        out=hw[:], out_offset=None, in_=h_dram[:],
                        in_offset=bass.IndirectOffsetOnAxis(ap=orow[:], axis=0))
                    tp = axp.tile([FEA, P], bf16, space="PSUM", tag="tp")
                    nc.tensor.transpose(out=tp[:], in_=hw[:], identity=ident[:])
                    hT = wp.tile([FEA, P], bf16, tag="hT")
                    nc.vector.tensor_copy(out=hT[:], in_=tp[:])
                    pa = axp.tile([P, P], f32, space="PSUM", tag="tp")
                    nc.tensor.matmul(out=pa[:], lhsT=hT[:],
                                     rhs=Wi_sb[:, bass.ds(lv, 128)],
                                     start=True, stop=True)
                    aw = wp.tile([P, P], bf16, tag="aw")
                    nc.vector.tensor_tensor(
                        out=aw[:], in0=pa[:],
                        in1=bias_sb[:, bass.ds(lv, 128)], op=OP.add)
                    nc.sync.dma_start(out=afs_dram[w * P:(w + 1) * P, :],
                                      in_=aw[:])

                # ---- B: edge pass (paired tiles) ----
                for w in range(NW):
                    srcw = wi_p.tile([P, TPW], i32, tag="srcw")
                    slotw = wi_p.tile([P, TPW], bf16, tag="slotw")
                    dstw = wi_p.tile([P, TPW], i32, tag="dstw")
                    nc.sync.dma_start(out=srcw[:], in_=src_in[w, :, :])
                    nc.sync.dma_start(out=slotw[:], in_=slot_in[w, :, :])
                    nc.sync.dma_start(out=dstw[:], in_=dstloc_in[w, :, :])
                    aggP = agp.tile([P, FEA], f32, space="PSUM", tag="agg")
                    eat = None
                    for pr in range(TPW // 2):
                        FS2 = fsp.tile([P, 2 * P], f32, space="PSUM", tag="FS")
                        ohs = []
                        for ti in range(2):
                            t = pr * 2 + ti
                            gt = w * TPW + t
                            if t % 4 == 0:
                                eat = wp.tile([EDGE_DIM, 4 * P], bf16,
                                              tag="eat")
                                nc.sync.dma_start(
                                    out=eat[:],
                                    in_=eaT_in[:, gt * P:(gt + 4) * P])
                            G = gp.tile([P, P], bf16, tag="G")
                            nc.gpsimd.indirect_dma_start(
                                out=G[:], out_offset=None, in_=bfs_all[:],
                                in_offset=bass.IndirectOffsetOnAxis(
                                    ap=srcw[:, t:t + 1], axis=0))
                            nc.gpsimd.indirect_dma_start(
                                out=G[:], out_offset=None, in_=afs_dram[:],
                                in_offset=bass.IndirectOffsetOnAxis(
                                    ap=dstw[:, t:t + 1], axis=0),
                                compute_op=OP.add)
                            nc.vector.tensor_copy(
                                out=FS2[:, ti * P:(ti + 1) * P], in_=G[:])
                            nc.tensor.matmul(
                                out=FS2[:, ti * P:(ti + 1) * P],
                                lhsT=eat[:, (t % 4) * P:(t % 4 + 1) * P],
                                rhs=Wea_sb[:, bass.ds(lv, 128)],
                                start=False, stop=True, skip_group_check=True)
                            oh = wp.tile([P, P], bf16, tag="oh")
                            nc.vector.tensor_tensor(
                                out=oh[:],
                                in0=slotw[:, t:t + 1].to_broadcast([P, P]),
                                in1=iota[:], op=OP.is_equal)
                            ohs.append(oh)
                        e1 = wp.tile([P, 2 * P], bf16, tag="e1")
                        nc.scalar.activation(out=e1[:], in_=FS2[:],
                                             func=AF.Exp)
                        l1 = wp.tile([P, 2 * P], bf16, tag="l1")
                        nc.scalar.activation(out=l1[:], in_=e1[:], func=AF.Ln,
                                             bias=1.0)
                        for ti in range(2):
                            t = pr * 2 + ti
                            sg = wp.tile([P, FEA], bf16, tag="sg")
                            nc.scalar.activation(
                                out=sg[:], in_=l1[:, ti * P:ti * P + FEA],
                                func=AF.Exp, scale=-1.0)
                            msg = wp.tile([P, FEA], bf16, tag="msg")
                            nc.vector.tensor_tensor(
                                out=msg[:], in0=sg[:],
                                in1=l1[:, ti * P + FEA:(ti + 1) * P],
                                op=OP.mult)
                            nc.tensor.matmul(
                                out=aggP[:], lhsT=ohs[ti][:], rhs=msg[:],
                                start=(t == 0), stop=(t == TPW - 1))
                    pay = wp.tile([P, FEA], bf16, tag="pay")
                    nc.vector.tensor_copy(out=pay[:], in_=aggP[:])
                    nc.sync.dma_start(out=agg_bounce[w * P:(w + 1) * P, :],
                                      in_=pay[:])

                # ---- C: collective ----
                nc.gpsimd.collective_compute(
                    "AllGather", OP.bypass, replica_groups=rg,
                    ins=[agg_bounce[:]], outs=[agg_all[:]])

                # ---- D: replicated BN + residual update ----
                statsP = a1p.tile([1, P], f32, space="PSUM", tag="stats")
                for ch in range(NCH):
                    a1 = wp.tile([P, FEA], bf16, tag="a4")
                    nc.sync.dma_start(out=a1[:],
                                      in_=agg_all[ch * P:(ch + 1) * P, :])
                    sq = wp.tile([P, FEA], bf16, tag="sq")
                    nc.vector.tensor_tensor(out=sq[:], in0=a1[:], in1=a1[:],
                                            op=OP.mult)
                    nc.tensor.matmul(out=statsP[:, 0:FEA],
                                     lhsT=mask_sb[:, ch:ch + 1], rhs=a1[:],
                                     start=(ch == 0), stop=(ch == NCH - 1),
                                     skip_group_check=True)
                    nc.tensor.matmul(out=statsP[:, FEA:P],
                                     lhsT=mask_sb[:, ch:ch + 1], rhs=sq[:],
                                     start=(ch == 0), stop=(ch == NCH - 1),
                                     skip_group_check=True)
                m1 = wp.tile([1, P], f32, tag="m1")
                nc.vector.tensor_scalar_mul(m1[:], statsP[:], 1.0 / N_NODES)
                mu2 = wp.tile([1, FEA], f32, tag="mu2")
                nc.vector.tensor_tensor(out=mu2[:], in0=m1[:, 0:FEA],
                                        in1=m1[:, 0:FEA], op=OP.mult)
                var = wp.tile([1, FEA], f32, tag="var")
                nc.vector.tensor_tensor(out=var[:], in0=m1[:, FEA:P],
                                        in1=mu2[:], op=OP.subtract)
                lnv = wp.tile([1, FEA], f32, tag="lnv")
                nc.scalar.activation(out=lnv[:], in_=var[:], func=AF.Ln,
                                     bias=eps_t[:])
                rstd = wp.tile([1, FEA], f32, tag="rstd")
                nc.scalar.activation(out=rstd[:], in_=lnv[:], func=AF.Exp,
                                     scale=-0.5)
                ssrow = wp.tile([1, P], f32, tag="ssrow")
                nc.vector.tensor_tensor(out=ssrow[:, 0:FEA], in0=rstd[:],
                                        in1=gam_sb[:, bass.ds(lv, 64)],
                                        op=OP.mult)
                msc = wp.tile([1, FEA], f32, tag="msc")
                nc.vector.tensor_tensor(out=msc[:], in0=m1[:, 0:FEA],
                                        in1=ssrow[:, 0:FEA], op=OP.mult)
                nc.vector.tensor_tensor(out=ssrow[:, FEA:P],
                                        in0=bet_sb[:, bass.ds(lv, 64)],
                                        in1=msc[:], op=OP.subtract)
                colp = axp.tile([P, 1], f32, space="PSUM", tag="tp")
                nc.tensor.transpose(out=colp[:], in_=ssrow[:],
                                    identity=identf[:1, :1])
                col = wp.tile([P, 1], f32, tag="col")
                nc.vector.tensor_copy(out=col[:], in_=colp[:])
                Mp = axp.tile([P, P], f32, space="PSUM", tag="tp")
                nc.tensor.transpose(out=Mp[:], in_=col[:].to_broadcast([P, P]),
                                    identity=identf[:])
                M = cp.tile([P, P], bf16, tag="M")
                nc.vector.tensor_copy(out=M[:], in_=Mp[:])
                for ch in range(NCH):
                    a1 = wp.tile([P, FEA], bf16, tag="a4")
                    nc.sync.dma_start(out=a1[:],
                                      in_=agg_all[ch * P:(ch + 1) * P, :])
                    t1 = wp.tile([P, FEA], bf16, tag="t1")
                    nc.vector.tensor_tensor(out=t1[:], in0=a1[:],
                                            in1=M[:, 0:FEA], op=OP.mult)
                    nc.vector.tensor_tensor(out=t1[:], in0=t1[:],
                                            in1=M[:, FEA:P], op=OP.add)
                    hsl = h_all[:, ch * FEA:(ch + 1) * FEA]
                    nc.vector.tensor_tensor(out=hsl, in0=t1[:], in1=hsl,
                                            op=OP.add)
                    nc.sync.dma_start(out=h_dram[ch * P:(ch + 1) * P, :],
                                      in_=hsl)

            # ================= pooling + head =================
            poolP = a1p.tile([P, FEA], f32, space="PSUM", tag="stats")
            cntP = agp.tile([P, 1], f32, space="PSUM", tag="agg")
            for ch in range(NCH):
                og = wp.tile([P, P], bf16, tag="og")
                nc.vector.tensor_tensor(
                    out=og[:], in0=batch_sb[:, ch:ch + 1].to_broadcast([P, P]),
                    in1=iota[:], op=OP.is_equal)
                nc.tensor.matmul(out=poolP[:], lhsT=og[:],
                                 rhs=h_all[:, ch * FEA:(ch + 1) * FEA],
                                 start=(ch == 0), stop=(ch == NCH - 1))
                nc.tensor.matmul(out=cntP[:], lhsT=og[:],
                                 rhs=mask_sb[:, ch:ch + 1],
                                 start=(ch == 0), stop=(ch == NCH - 1))
            cnt = wp.tile([P, 1], f32, tag="cnt_s")
            nc.vector.tensor_scalar_max(cnt[:], cntP[:], 1.0)
            rec = wp.tile([P, 1], f32, tag="rec")
            nc.vector.reciprocal(rec[:], cnt[:])
            pooled = wp.tile([P, FEA], bf16, tag="pooled")
            nc.vector.tensor_scalar_mul(pooled[:], poolP[:], rec[:])

            def head_mm(in_bf, k, n, W_list, bmat, act):
                outp = fsp.tile([P, n], f32, space="PSUM", tag="FS")
                nchunks = (k + P - 1) // P
                for i in range(nchunks):
                    kk = min(P, k - i * P)
                    tp = axp.tile([P, P], bf16, space="PSUM", tag="tp")
                    nc.tensor.transpose(out=tp[:kk, :],
                                        in_=in_bf[:, i * P:i * P + kk],
                                        identity=ident[:])
                    tT = wp.tile([P, P], bf16, tag="hT")
                    nc.vector.tensor_copy(out=tT[:kk, :], in_=tp[:kk, :])
                    nc.tensor.matmul(out=outp[:], lhsT=tT[:kk, :],
                                     rhs=W_list[i][:kk, :n],
                                     start=(i == 0), stop=(i == nchunks - 1))
                zb = wp.tile([P, n], f32, tag="zb")
                nc.vector.tensor_tensor(out=zb[:], in0=outp[:],
                                        in1=bmat[:, :n], op=OP.add)
                if not act:
                    return zb
                ez = wp.tile([P, n], bf16, tag="ez")
                nc.scalar.activation(out=ez[:], in_=zb[:], func=AF.Exp)
                g = wp.tile([P, n], bf16, tag="g")
                nc.scalar.activation(out=g[:], in_=ez[:], func=AF.Ln, bias=1.0)
                return g

            g1 = head_mm(pooled, FEA, HID, [W1_sb], b1_sb, True)
            g2 = head_mm(g1, HID, HID, [W2a_sb, W2b_sb], b2_sb, True)
            dosv = head_mm(g2, HID, LATENT * CH, [W3a_sb, W3b_sb], b3_sb,
                           False)
            nc.sync.dma_start(out=dos_out[:], in_=dosv[:])

            hd = wp.tile([P, FEA], f32, tag="hdbg")
            nc.vector.tensor_copy(out=hd[:], in_=h_all[:, 0:FEA])
            nc.sync.dma_start(out=hdbg_out[:], in_=hd[:])

    nc.compile()
    return nc


# ======================== host side ========================

def _prepare(x, edge_index, edge_attr, batch):
    deg = np.bincount(edge_index[1].astype(np.int64), minlength=N_NODES)
    nbins = NCORES * NW
    order = np.argsort(-deg, kind="stable")
    nrounds = (N_NODES + nbins - 1) // nbins
    pad = nrounds * nbins - N_NODES
    padded = np.concatenate([order, np.full(pad, -1, np.int64)])
    grid = padded.reshape(nrounds, nbins)
    grid[1::2] = grid[1::2, ::-1]
    bin_of = np.full(N_NODES, -1, np.int64)
    slot_of = np.full(N_NODES, -1, np.int64)
    for r in range(nrounds):
        row = grid[r]
        v = row >= 0
        bin_of[row[v]] = np.nonzero(v)[0]
        slot_of[row[v]] = r
    assert nrounds <= P
    newid = (bin_of // NW) * VC + (bin_of % NW) * P + slot_of

    bin_edges = np.bincount(bin_of[edge_index[1].astype(np.int64)],
                            minlength=nbins)
    assert bin_edges.max() <= TPW * P, f"window overflow {bin_edges.max()}"

    src_new = newid[edge_index[0].astype(np.int64)].astype(np.int32)
    dst_new = newid[edge_index[1].astype(np.int64)].astype(np.int32)
    gwin = dst_new // P
    eorder = np.argsort(gwin, kind="stable")
    gwin_s = gwin[eorder]
    src_s = src_new[eorder]
    dst_s = dst_new[eorder]
    ea_s = np.asarray(edge_attr)[eorder]

    counts = np.bincount(gwin_s, minlength=nbins)
    offs = np.concatenate([[0], np.cumsum(counts)])
    within = np.arange(len(gwin_s)) - offs[gwin_s]
    tgt = (gwin_s // NW) * EC + (gwin_s % NW) * (TPW * P) + within

    src_pad = np.zeros(NCORES * EC, np.int32)
    dstloc_pad = np.zeros(NCORES * EC, np.int32)
    slot_pad = np.full(NCORES * EC, -1.0, np.float32)
    ea_pad = np.zeros((NCORES * EC, EDGE_DIM), np.float32)
    src_pad[tgt] = src_s
    dstloc_pad[tgt] = (dst_s % VC).astype(np.int32)
    slot_pad[tgt] = (dst_s % P).astype(np.float32)
    ea_pad[tgt] = ea_s

    # device edge coords: within-window slot s -> partition p = s // TPW,
    # tile t = s % TPW.  srcw/slotw tiles are [P, TPW].
    src_dev = src_pad.reshape(NCORES, NW, P, TPW)
    dstloc_dev = dstloc_pad.reshape(NCORES, NW, P, TPW)
    slot_dev = slot_pad.reshape(NCORES, NW, P, TPW).astype(ml_dtypes.bfloat16)
    # eaT column for tile gt=w*TPW+t, partition p is gt*P + p
    ea_dev = ea_pad.reshape(NCORES, NW, P, TPW, EDGE_DIM)
    ea_cols = ea_dev.transpose(0, 1, 3, 2, 4).reshape(NCORES, EC, EDGE_DIM)
    eaT = np.ascontiguousarray(ea_cols.transpose(0, 2, 1)).astype(
        ml_dtypes.bfloat16)

    xT = np.zeros((NCORES, ATOM_IN, VC), np.float32)
    old_of_new = np.full(VTOT, -1, np.int64)
    old_of_new[newid] = np.arange(N_NODES)
    xs = np.asarray(x)
    bs = np.asarray(batch).astype(np.int64)
    for c in range(NCORES):
        sel = old_of_new[c * VC:(c + 1) * VC]
        v = sel >= 0
        xT[c][:, v] = xs[sel[v]].T

    vn = (old_of_new >= 0).reshape(NCH, P)
    oo = old_of_new.reshape(NCH, P)
    bc = np.full((NCH, P), 500.0, np.float32)
    bc[vn] = bs[oo[vn]].astype(np.float32)
    batch_col = np.ascontiguousarray(bc.T).astype(ml_dtypes.bfloat16)
    mask_col = np.ascontiguousarray(vn.T.astype(np.float32)).astype(
        ml_dtypes.bfloat16)

    ownrows = np.zeros((NCORES, NW, P), np.int32)
    maskown = np.zeros((NCORES, P, NW), np.float32)
    for c in range(NCORES):
        rows = c * VC + np.arange(VC)
        ownrows[c] = rows.reshape(NW, P)
        maskown[c] = vn[c * NW:(c + 1) * NW].T.astype(np.float32)

    return dict(src=src_dev, dstloc=dstloc_dev, slot=slot_dev, eaT=eaT,
                xT=xT.astype(ml_dtypes.bfloat16), batch_col=batch_col,
                mask_col=mask_col, ownrows=ownrows,
                maskown=maskown.astype(ml_dtypes.bfloat16), newid=newid)


def _prep_weights(ins):
    def neg_f(w):
        w = np.array(w, np.float32)
        w[..., :FEA] = -w[..., :FEA]
        return w

    Wi_s = np.zeros((64, N_CONV * 256), np.float32)
    Wj_s = np.zeros((64, N_CONV * 256), np.float32)
    Wea_s = np.zeros((EDGE_DIM, N_CONV * 256), np.float32)
    bias_s = np.zeros((P, N_CONV * 256), np.float32)
    gam_s = np.zeros((1, N_CONV * 256), np.float32)
    bet_s = np.zeros((1, N_CONV * 256), np.float32)
    for i in range(N_CONV):
        Wf = np.asarray(ins["lin_f_W"][i])
        Ws = np.asarray(ins["lin_s_W"][i])
        bfv = np.asarray(ins["lin_f_b"][i])
        bsv = np.asarray(ins["lin_s_b"][i])
        Wi_s[:, i * 256:i * 256 + 128] = neg_f(
            np.concatenate([Wf[0:64], Ws[0:64]], axis=1))
        Wj_s[:, i * 256:i * 256 + 128] = neg_f(
            np.concatenate([Wf[64:128], Ws[64:128]], axis=1))
        Wea_s[:, i * 256:i * 256 + 128] = neg_f(
            np.concatenate([Wf[128:169], Ws[128:169]], axis=1))
        bias_s[:, i * 256:i * 256 + 128] = np.broadcast_to(
            neg_f(np.concatenate([bfv, bsv])), (P, 128))
        gam_s[0, i * 256:i * 256 + 64] = np.asarray(ins["bn_gamma"][i])
        bet_s[0, i * 256:i * 256 + 64] = np.asarray(ins["bn_beta"][i])
    return Wi_s, Wj_s, Wea_s, bias_s, gam_s, bet_s


def kernel(**inputs):
    x = np.asarray(inputs["x"], np.float32)
    edge_index = np.asarray(inputs["edge_index"])
    edge_attr = np.asarray(inputs["edge_attr"], np.float32)
    batch = np.asarray(inputs["batch"])

    prep = _prepare(x, edge_index, edge_attr, batch)
    Wi_s, Wj_s, Wea_s, bias_s, gam_s, bet_s = _prep_weights(inputs)

    bf = ml_dtypes.bfloat16
    iota = np.broadcast_to(np.arange(P, dtype=np.float32), (P, P)).astype(bf)
    embB = np.broadcast_to(np.asarray(inputs["emb_b"], np.float32), (P, FEA))
    b1 = np.broadcast_to(np.asarray(inputs["head_b1"], np.float32), (P, HID))
    b2 = np.broadcast_to(np.asarray(inputs["head_b2"], np.float32), (P, HID))
    b3 = np.broadcast_to(np.asarray(inputs["head_b3"], np.float32),
                         (P, LATENT * CH))

    if "nc" not in _cache:
        _cache["nc"] = _build()
    nc = _cache["nc"]

    in_maps = []
    for c in range(NCORES):
        in_maps.append(dict(
            xT_in=np.ascontiguousarray(prep["xT"][c]),
            src_in=np.ascontiguousarray(prep["src"][c]),
            slot_in=np.ascontiguousarray(prep["slot"][c]),
            dstloc_in=np.ascontiguousarray(prep["dstloc"][c]),
            eaT_in=np.ascontiguousarray(prep["eaT"][c]),
            ownrows_in=np.ascontiguousarray(prep["ownrows"][c]),
            maskown_in=np.ascontiguousarray(prep["maskown"][c]),
            batch_in=prep["batch_col"], mask_in=prep["mask_col"],
            iota_in=np.asarray(iota),
            embW_in=np.asarray(inputs["emb_W"], np.float32).astype(bf),
            embB_in=np.ascontiguousarray(embB),
            Wi_in=Wi_s.astype(bf), Wj_in=Wj_s.astype(bf),
            Wea_in=Wea_s.astype(bf), bias_in=bias_s.astype(bf),
            gam_in=gam_s, bet_in=bet_s,
            W1_in=np.asarray(inputs["head_W1"], np.float32).astype(bf),
            W2_in=np.asarray(inputs["head_W2"], np.float32).astype(bf),
            W3_in=np.asarray(inputs["head_W3"], np.float32).astype(bf),
            b1_in=np.ascontiguousarray(b1), b2_in=np.ascontiguousarray(b2),
            b3_in=np.ascontiguousarray(b3),
        ))
    res = run_bass_kernel_spmd(nc, in_maps, list(range(NCORES)))
    dos = res.results[0]["dos_out"].astype(np.float32)
    _cache["hdbg"] = res.results[0]["hdbg_out"]
    _cache["newid"] = prep["newid"]
    return dos.reshape(NUM_GRAPHS, CH, LATENT)

